# revision 1
# baseline (speedup 1.0000x reference)
"""Causal single-head attention (B=4, S=2048, D=768) on 8 TRN2 NeuronCores.

Sharding: core (b, h) = batch b, sequence-half h. Each core computes the
attention output for 1024 query rows of one batch. Keys are fed ROTATED by
h*1024 so every core sees the identical score structure: a causal triangle
over the first 1024 key columns plus a dense block over the last 1024 key
columns that is alive only for h=1 (killed via the exp bias input for h=0).

Per-core dataflow (all big matmuls in float32r, 1 cycle/row at N>=256):
  QT[e,i] / KT[e,j] from pre-transposed (host-side) xT and wT inputs,
  V[j,e]; scores per 128-row query tile; exp (+1/sqrt(d) scale and row-sum)
  fused in one ScalarE activation; P tiles transposed on the PE; PV
  accumulated in PSUM; final divide by the row-sum on the way out.
"""

import os
import numpy as np

import concourse.bass as bass
import concourse.mybir as mybir
import concourse.tile as tile
from concourse import bacc
from concourse.bass_utils import run_bass_kernel_spmd

B, S, D = 4, 2048, 768
H = S // 2           # query rows per core
P = 128
ND = D // P          # 6  d/e tiles
NQ = H // P          # 8  query tiles per core
NK = S // P          # 16 key tiles
SCALE = 1.0 / float(np.sqrt(D))
NEG = -10000.0
F32 = mybir.dt.float32
F32R = mybir.dt.float32r

_cached = {}
last_results = None


def _build_nc():
    nc = bacc.Bacc("TRN2", target_bir_lowering=False)

    xT_d = nc.dram_tensor("xT", [D, S], F32, kind="ExternalInput")
    wqT_d = nc.dram_tensor("wqT", [D, D], F32, kind="ExternalInput")
    wkT_d = nc.dram_tensor("wkT", [D, D], F32, kind="ExternalInput")
    wvT_d = nc.dram_tensor("wvT", [D, D], F32, kind="ExternalInput")
    fb_d = nc.dram_tensor("fbias", [P, 1], F32, kind="ExternalInput")
    out_d = nc.dram_tensor("out", [H, D], F32, kind="ExternalOutput")

    with tile.TileContext(nc) as tc:
        with (
            tc.tile_pool(name="qtp", bufs=ND) as qtp,
            tc.tile_pool(name="ktp", bufs=ND) as ktp,
            tc.tile_pool(name="vp", bufs=NK) as vp,
            tc.tile_pool(name="cst", bufs=1) as cst,
        ):
            fb = cst.tile([P, 1], F32)
            nc.sync.dma_start(out=fb[:], in_=fb_d[:, :])

            qts, kts, vs = [], [], []
            # ---- projections (xT/w pools scoped so their SBUF frees) ----
            with (
                tc.tile_pool(name="xp", bufs=ND) as xp,
                tc.tile_pool(name="wp", bufs=7) as wp,
                tc.tile_pool(name="psj", bufs=8, space="PSUM") as psj,
            ):
                def load_w(w_dram):
                    tiles = []
                    for d in range(ND):
                        wt = wp.tile([P, D], F32R, tag="w")
                        nc.sync.dma_start(out=wt[:], in_=w_dram[d * P:(d + 1) * P, :].bitcast(F32R))
                        tiles.append(wt)
                    return tiles

                # interleave w_k with the first x column-chunk (d-paired) so
                # K-proj accumulation trickles with DMA arrival; stream the
                # remaining x chunks column-major
                xs = [xp.tile([P, S], F32R, name=f"xt{d}", tag="xt") for d in range(ND)]
                wk = []
                for d in range(ND):
                    wt = wp.tile([P, D], F32R, tag="w")
                    nc.sync.dma_start(out=wt[:], in_=wkT_d[d * P:(d + 1) * P, :].bitcast(F32R))
                    wk.append(wt)
                    nc.sync.dma_start(
                        out=xs[d][:, 0:512],
                        in_=xT_d[d * P:(d + 1) * P, 0:512].bitcast(F32R))
                for c0 in range(512, S, 512):
                    for d in range(ND):
                        nc.sync.dma_start(
                            out=xs[d][:, c0:c0 + 512],
                            in_=xT_d[d * P:(d + 1) * P, c0:c0 + 512].bitcast(F32R))

                # KT[e,j] = sum_d wkT[d,e]^T xT[d,j]
                for et in range(ND):
                    kt = ktp.tile([P, S], F32R)
                    kts.append(kt)
                    for c0 in range(0, S, 512):
                        acc = psj.tile([P, 512], F32, tag="ps")
                        for d in range(ND):
                            nc.tensor.matmul(
                                acc[:],
                                wk[d][:, et * P:(et + 1) * P],
                                xs[d][:, c0:c0 + 512],
                                start=(d == 0), stop=(d == ND - 1),
                            )
                        nc.vector.tensor_copy(kt[:, c0:c0 + 512], acc[:])

                # V[j,e] = sum_d xT[d,j]^T wvT[d,e]
                wv = load_w(wvT_d)
                for jt in range(NK):
                    v = vp.tile([P, D + 2], F32R)
                    vs.append(v)
                    for e0, ew in ((0, 512), (512, 256)):
                        acc = psj.tile([P, 512], F32, tag="ps")
                        for d in range(ND):
                            nc.tensor.matmul(
                                acc[:, :ew],
                                xs[d][:, jt * P:(jt + 1) * P],
                                wv[d][:, e0:e0 + ew],
                                start=(d == 0), stop=(d == ND - 1),
                            )
                        nc.vector.tensor_copy(v[:, e0:e0 + ew], acc[:, :ew])
                    ones = nc.const_aps.tensor(1.0, (P, 2), F32)
                    nc.vector.tensor_copy(v[:, D:D + 2], ones)

                # QT[e,i] = sum_d wqT[d,e]^T xT[d,i]  for i in [0, H)
                wq = load_w(wqT_d)
                for et in range(ND):
                    qt = qtp.tile([P, H], F32R)
                    qts.append(qt)
                    for qc0, xc0 in ((0, 0), (512, 1024)):
                        acc = psj.tile([P, 512], F32, tag="ps")
                        for d in range(ND):
                            nc.tensor.matmul(
                                acc[:],
                                wq[d][:, et * P:(et + 1) * P],
                                xs[d][:, xc0:xc0 + 512],
                                start=(d == 0), stop=(d == ND - 1),
                            )
                        nc.vector.tensor_copy(qt[:, qc0:qc0 + 512], acc[:])

            # ---- attention: transposed scores over 512-query quads ----
            # scoresT[j, i] tiles [128, 512] (four query tiles per pass; N=512
            # amortizes the per-matmul LDWEIGHTS, which is the PE limiter at
            # N=256). Phase A: scores + exp into PT tiles for every live key
            # tile of the quad. Phase B: PV per 128-query half. The ones
            # columns of V give the softmax denominator in pv[:, D].
            with (
                tc.tile_pool(name="ptp", bufs=24) as ptp,
                tc.tile_pool(name="dgp", bufs=3) as dgp,
                tc.tile_pool(name="sgp", bufs=4) as sgp,
                tc.tile_pool(name="op", bufs=3) as op,
                tc.tile_pool(name="ps", bufs=4, space="PSUM") as ps_pool,
                tc.tile_pool(name="pspv", bufs=2, space="PSUM") as pspv_pool,
            ):
                for q in range(2):
                    qc = 512 * q              # QT column base
                    i_rot0 = 0 if q == 0 else 1024
                    if q == 0:
                        jts = list(range(4)) + list(range(12, NK))
                        tri = set(range(4))
                    else:
                        jts = list(range(12)) + list(range(12, NK))
                        tri = set(range(8, 12))
                    pts = {}
                    for jt in jts:
                        st = ps_pool.tile([P, 512], F32, tag="st")
                        for et in range(ND):
                            nc.tensor.matmul(
                                st[:],
                                kts[et][:, jt * P:(jt + 1) * P],
                                qts[et][:, qc:qc + 512],
                                start=(et == 0), stop=(et == ND - 1),
                            )
                        pt = ptp.tile([P, 512], F32R, tag="pt")
                        pts[jt] = pt
                        if jt in tri:
                            dg = dgp.tile([P, 512], F32, tag="dg")
                            nc.vector.tensor_copy(dg[:], st[:])
                            # keep where (i_rot0 + f) - (128*jt + p) >= 0
                            nc.gpsimd.affine_select(
                                out=dg[:], in_=dg[:],
                                compare_op=mybir.AluOpType.is_ge,
                                fill=NEG, base=i_rot0 - P * jt,
                                pattern=[[1, 512]], channel_multiplier=-1,
                            )
                            nc.scalar.activation(
                                pt[:], dg[:], mybir.ActivationFunctionType.Exp,
                                bias=0.0, scale=SCALE,
                            )
                        else:
                            nc.scalar.activation(
                                pt[:], st[:], mybir.ActivationFunctionType.Exp,
                                bias=(fb[:, 0:1] if jt >= 12 else 0.0), scale=SCALE,
                            )
                    for half in range(4):
                        pv = pspv_pool.tile([P, D + 2], F32, tag="pv")
                        h0 = half * P
                        for idx, jt in enumerate(jts):
                            for e0, ew in ((0, 512), (512, D + 2 - 512)):
                                nc.tensor.matmul(
                                    pv[:, e0:e0 + ew],
                                    pts[jt][:, h0:h0 + P],
                                    vs[jt][:, e0:e0 + ew],
                                    start=(idx == 0), stop=(idx == len(jts) - 1),
                                )
                        rcp = sgp.tile([P, 1], F32, tag="rcp")
                        nc.vector.reciprocal(rcp[:], pv[:, D:D + 1])
                        o = op.tile([P, D], F32, tag="o")
                        nc.vector.tensor_scalar_mul(o[:], pv[:, :D], rcp[:])
                        r0 = qc + h0
                        nc.sync.dma_start(out=out_d[r0:r0 + P, :], in_=o[:])

    nc.compile()
    return nc


def _get_nc():
    if "nc" not in _cached:
        _cached["nc"] = _build_nc()
    return _cached["nc"]


def kernel(x, w_q, w_k, w_v):
    global last_results
    x = np.ascontiguousarray(np.asarray(x, dtype=np.float32))
    wqT = np.ascontiguousarray(np.asarray(w_q, dtype=np.float32).T)
    wkT = np.ascontiguousarray(np.asarray(w_k, dtype=np.float32).T)
    wvT = np.ascontiguousarray(np.asarray(w_v, dtype=np.float32).T)

    nc = _get_nc()
    in_maps = []
    for core in range(8):
        b, h = core // 2, core % 2
        r = 512 * h
        rot = np.concatenate([x[b, r:], x[b, :r]], axis=0)
        in_maps.append({
            "xT": np.ascontiguousarray(rot.T),
            "wqT": wqT, "wkT": wkT, "wvT": wvT,
            "fbias": np.full((P, 1), 0.0 if h == 1 else NEG, np.float32),
        })

    trace = bool(int(os.environ.get("KERNEL_TRACE", "0")))
    res = run_bass_kernel_spmd(nc, in_maps, core_ids=list(range(8)), trace=trace)
    last_results = res

    out = np.empty((B, S, D), np.float32)
    for core in range(8):
        b, h = core // 2, core % 2
        r = 512 * h
        o = res.results[core]["out"]
        out[b, r:r + 512] = o[0:512]
        out[b, 1024 + r:1024 + r + 512] = o[512:1024]
    return out



# revision 6
# speedup vs baseline: 1.5478x; 1.5478x over previous
"""Causal single-head attention (B=4, S=2048, D=768) on 8 TRN2 NeuronCores.

Sharding: core (b, h) = batch b, query-interleave h. Each core computes the
attention output for query tiles {2k+h : k=0..7} (128 rows each) of one
batch. Keys are fed ROTATED by 128*h so every core sees the identical score
structure: query tile k sits at rotated row 256k and attends rotated key
tiles 0..2k (tile 2k triangular) plus the wrap tile 15, which holds the
original first 128 keys for h=1 and is killed via the exp bias for h=0.
This balances causal work exactly across the core pair.

QK merge: scores = x (W_q^T W_k) x^T, with M = W_q^T W_k computed on the
host. The kernel computes TT = M^T xq^T (one projection instead of Q and K)
and uses the resident x tiles directly as the score stationaries, removing
the K projection from the device entirely.

All matmul inputs are bf16 (1 cycle/row on the PE, same as f32r, but half
the DMA bytes); accumulation stays f32 in PSUM. Softmax denominators come
from two ones-columns appended to V (cols 768..769), divided out on the way
to the output.
"""

import os
import numpy as np
import ml_dtypes

import concourse.bass as bass
import concourse.mybir as mybir
import concourse.tile as tile
from concourse import bacc
from concourse.bass_utils import run_bass_kernel_spmd

B, S, D = 4, 2048, 768
P = 128
ND = D // P          # 6 contraction tiles
NQT = 8              # query tiles per core (128 rows each)
H = NQT * P          # 1024 query rows per core
NK = S // P          # 16 key tiles
SCALE = 1.0 / float(np.sqrt(D))
NEG = -10000.0
F32 = mybir.dt.float32
BF16 = mybir.dt.bfloat16
BF = ml_dtypes.bfloat16

_cached = {}
last_results = None


def _k0(j):
    # first query tile whose score group includes key tile j (j < 15);
    # clamped to 6 so every group spans >= 256 moving columns
    return min((j + 1) // 2, 6)


def _scores_phase(nc, tc, fb, xs, tts, ptp, dgp, pss):
    pts = {}
    for j in [15] + list(range(15)):
        k0 = 0 if j == 15 else _k0(j)
        ncol = (NQT - k0) * P
        pt = ptp.tile([P, H], BF16)
        pts[j] = pt
        for qoff in range(0, ncol, 512):
            qw = min(512, ncol - qoff)
            st = pss.tile([P, 512], F32, tag="st")
            for dp in range(ND):
                nc.tensor.matmul(
                    st[:, :qw],
                    xs[dp][:, j * P:(j + 1) * P],
                    tts[dp][:, k0 * P + qoff:k0 * P + qoff + qw],
                    start=(dp == 0), stop=(dp == ND - 1),
                )
            if j == 15:
                nc.scalar.activation(
                    pt[:, qoff:qoff + qw], st[:, :qw],
                    mybir.ActivationFunctionType.Exp,
                    bias=fb[:, 0:1], scale=SCALE,
                )
            elif qoff == 0:
                dg = dgp.tile([P, 256], F32, tag="dg")
                nc.vector.tensor_copy(dg[:], st[:, 0:256])
                # keep where 256*k0 + 256*kk + f - (128*j + p) >= 0
                nc.gpsimd.affine_select(
                    out=dg[:], in_=dg[:],
                    compare_op=mybir.AluOpType.is_ge,
                    fill=NEG, base=256 * k0 - P * j,
                    pattern=[[256, 2], [1, P]], channel_multiplier=-1,
                )
                nc.scalar.activation(
                    pt[:, 0:256], dg[:],
                    mybir.ActivationFunctionType.Exp,
                    bias=0.0, scale=SCALE,
                )
                if qw > 256:
                    nc.scalar.activation(
                        pt[:, 256:qw], st[:, 256:qw],
                        mybir.ActivationFunctionType.Exp,
                        bias=0.0, scale=SCALE,
                    )
            else:
                nc.scalar.activation(
                    pt[:, qoff:qoff + qw], st[:, :qw],
                    mybir.ActivationFunctionType.Exp,
                    bias=0.0, scale=SCALE,
                )
    return pts


def _build_nc():
    nc = bacc.Bacc("TRN2", target_bir_lowering=False)

    m_d = nc.dram_tensor("m", [D, D], BF16, kind="ExternalInput")
    xqT_d = nc.dram_tensor("xqT", [D, H], BF16, kind="ExternalInput")
    xT_d = nc.dram_tensor("xT", [D, S], BF16, kind="ExternalInput")
    wvT_d = nc.dram_tensor("wvT", [D, D], BF16, kind="ExternalInput")
    fb_d = nc.dram_tensor("fbias", [P, 1], F32, kind="ExternalInput")
    out_d = nc.dram_tensor("out", [H, D], F32, kind="ExternalOutput")

    with tile.TileContext(nc) as tc:
        with (
            tc.tile_pool(name="cst", bufs=1) as cst,
            tc.tile_pool(name="xp", bufs=ND) as xp,
            tc.tile_pool(name="ttp", bufs=ND) as ttp,
            tc.tile_pool(name="vp", bufs=NK) as vp,
            tc.tile_pool(name="ptp", bufs=NK) as ptp,
            tc.tile_pool(name="dgp", bufs=2) as dgp,
            tc.tile_pool(name="sgp", bufs=2) as sgp,
            tc.tile_pool(name="op", bufs=2) as op,
        ):
            fb = cst.tile([P, 1], F32)
            nc.sync.dma_start(out=fb[:], in_=fb_d[:, :])

            tts, vs, xs = [], [], []
            # ---- projections (m/xq/wv pools scoped so their SBUF+PSUM free) ----
            with (
                tc.tile_pool(name="mp", bufs=ND) as mp,
                tc.tile_pool(name="xqp", bufs=ND) as xqp,
                tc.tile_pool(name="wvp", bufs=ND) as wvp,
                tc.tile_pool(name="psj", bufs=4, space="PSUM") as psj,
            ):
                # DMA: M + first xq halves (unblock TT), rest of xq,
                # then wv, then x key tiles
                ms, xqs, wv = [], [], []
                for d in range(ND):
                    mt = mp.tile([P, D], BF16)
                    nc.sync.dma_start(out=mt[:], in_=m_d[d * P:(d + 1) * P, :])
                    ms.append(mt)
                    xq = xqp.tile([P, H], BF16)
                    nc.sync.dma_start(out=xq[:, 0:512], in_=xqT_d[d * P:(d + 1) * P, 0:512])
                    xqs.append(xq)
                for d in range(ND):
                    nc.sync.dma_start(out=xqs[d][:, 512:H], in_=xqT_d[d * P:(d + 1) * P, 512:H])
                for d in range(ND):
                    wt = wvp.tile([P, D], BF16)
                    nc.sync.dma_start(out=wt[:], in_=wvT_d[d * P:(d + 1) * P, :])
                    wv.append(wt)
                for d in range(ND):
                    xt = xp.tile([P, S], BF16, name=f"xt{d}", tag="xt")
                    xs.append(xt)
                for c0 in range(0, S, 512):
                    for d in range(ND):
                        nc.sync.dma_start(
                            out=xs[d][:, c0:c0 + 512],
                            in_=xT_d[d * P:(d + 1) * P, c0:c0 + 512])

                # TT[d', i] = sum_d M[d, d'] xq^T[d, i]
                for et in range(ND):
                    tt = ttp.tile([P, H], BF16)
                    tts.append(tt)
                    for qc in (0, 512):
                        acc = psj.tile([P, 512], F32, tag="ps")
                        for d in range(ND):
                            nc.tensor.matmul(
                                acc[:],
                                ms[d][:, et * P:(et + 1) * P],
                                xqs[d][:, qc:qc + 512],
                                start=(d == 0), stop=(d == ND - 1),
                            )
                        nc.vector.tensor_copy(tt[:, qc:qc + 512], acc[:])

                # V[j, e] = sum_d x^T[d, j] wv^T[d, e]; ones in cols 768..769
                ones = nc.const_aps.tensor(1.0, (P, 2), F32)
                for j in range(NK):
                    v = vp.tile([P, D + 2], BF16)
                    vs.append(v)
                    for e0, ew in ((0, 512), (512, 256)):
                        acc = psj.tile([P, 512], F32, tag="ps")
                        for d in range(ND):
                            nc.tensor.matmul(
                                acc[:, :ew],
                                xs[d][:, j * P:(j + 1) * P],
                                wv[d][:, e0:e0 + ew],
                                start=(d == 0), stop=(d == ND - 1),
                            )
                        nc.vector.tensor_copy(v[:, e0:e0 + ew], acc[:, :ew])
                    nc.vector.tensor_copy(v[:, D:D + 2], ones)

            # ---- attention ----
            # scores^T[j', i] per key tile j: moving = TT[:, 128*k0 :] (query
            # suffix), stationary = x key tile j. Wrap tile 15 first (needed
            # by every PV chain); its exp is gated by fbias. Other groups get
            # one affine_select over the first 256 columns killing the
            # diagonal triangle and any dead leading tile. Then PV per query
            # tile k accumulates over j in {15, 0..2k}; the ones-columns of V
            # give the softmax denominator in pv[:, 768].
            with (
                tc.tile_pool(name="pss", bufs=2, space="PSUM") as pss,
                tc.tile_pool(name="pspv", bufs=2, space="PSUM") as pspv,
            ):
                pts = _scores_phase(nc, tc, fb, xs, tts, ptp, dgp, pss)

                for k in range(NQT):
                    pv = pspv.tile([P, D + 2], F32, tag="pv")
                    js = [15] + list(range(2 * k + 1))
                    for idx, j in enumerate(js):
                        k0 = 0 if j == 15 else _k0(j)
                        koff = (k - k0) * P
                        for e0, ew in ((0, 512), (512, D + 2 - 512)):
                            nc.tensor.matmul(
                                pv[:, e0:e0 + ew],
                                pts[j][:, koff:koff + P],
                                vs[j][:, e0:e0 + ew],
                                start=(idx == 0), stop=(idx == len(js) - 1),
                            )
                    rcp = sgp.tile([P, 1], F32, tag="rcp")
                    nc.vector.reciprocal(rcp[:], pv[:, D:D + 1])
                    o = op.tile([P, D], F32, tag="o")
                    nc.vector.tensor_scalar_mul(o[:], pv[:, :D], rcp[:])
                    nc.sync.dma_start(out=out_d[k * P:(k + 1) * P, :], in_=o[:])

    nc.compile()
    return nc


def _get_nc():
    if "nc" not in _cached:
        _cached["nc"] = _build_nc()
    return _cached["nc"]


def kernel(x, w_q, w_k, w_v):
    global last_results
    x = np.ascontiguousarray(np.asarray(x, dtype=np.float32))
    w_q = np.asarray(w_q, dtype=np.float32)
    w_k = np.asarray(w_k, dtype=np.float32)
    w_v = np.asarray(w_v, dtype=np.float32)

    m = np.ascontiguousarray(w_q.T @ w_k).astype(BF)
    wvT = np.ascontiguousarray(w_v.T).astype(BF)

    nc = _get_nc()
    in_maps = []
    for core in range(8):
        b, h = core // 2, core % 2
        r = P * h
        rot = np.concatenate([x[b, r:], x[b, :r]], axis=0)
        xq = x[b].reshape(NK, P, D)[h::2].reshape(H, D)
        in_maps.append({
            "m": m,
            "xqT": np.ascontiguousarray(xq.T).astype(BF),
            "xT": np.ascontiguousarray(rot.T).astype(BF),
            "wvT": wvT,
            "fbias": np.full((P, 1), 0.0 if h == 1 else NEG, np.float32),
        })

    trace = bool(int(os.environ.get("KERNEL_TRACE", "0")))
    res = run_bass_kernel_spmd(nc, in_maps, core_ids=list(range(8)), trace=trace)
    last_results = res

    out = np.empty((B, S, D), np.float32)
    for core in range(8):
        b, h = core // 2, core % 2
        o = res.results[core]["out"]
        out[b].reshape(NK, P, D)[h::2] = o.reshape(NQT, P, D)
    return out


# revision 12
# speedup vs baseline: 1.7960x; 1.1604x over previous
"""Causal single-head attention (B=4, S=2048, D=768) on 8 TRN2 NeuronCores.

Sharding: core (b, h) = batch b, query-interleave h. Each core computes the
attention output for query tiles {2k+h : k=0..7} (128 rows each) of one
batch. Keys are fed ROTATED by 128*h so every core sees the identical score
structure: query tile k sits at rotated row 256k and attends rotated key
tiles 0..2k (tile 2k triangular) plus the wrap tile 15, which holds the
original first 128 keys for h=1 and is killed via the exp bias for h=0.
This balances causal work exactly across the core pair.

QK merge: scores = x (W_q^T W_k) x^T, with M = W_q^T W_k computed on the
host. The kernel computes TT = M^T xq^T (one projection instead of Q and K)
and uses the resident x tiles directly as the score stationaries, removing
the K projection from the device entirely.

All matmul inputs are bf16 (1 cycle/row on the PE, same as f32r, but half
the DMA bytes); accumulation stays f32 in PSUM. Softmax denominators come
from two ones-columns appended to V (cols 768..769), divided out on the way
to the output.
"""

import os
import numpy as np
import ml_dtypes

import concourse.bass as bass
import concourse.mybir as mybir
import concourse.tile as tile
from concourse import bacc
from concourse.bass_utils import run_bass_kernel_spmd

B, S, D = 4, 2048, 768
P = 128
ND = D // P          # 6 contraction tiles
NQT = 8              # query tiles per core (128 rows each)
H = NQT * P          # 1024 query rows per core
NK = S // P          # 16 key tiles
SCALE = 1.0 / float(np.sqrt(D))
NEG = -10000.0
F32 = mybir.dt.float32
BF16 = mybir.dt.bfloat16
BF = ml_dtypes.bfloat16

_cached = {}
last_results = None


def _k0(j):
    # first query tile whose score group includes key tile j (j < 15);
    # clamped to 6 so every group spans >= 256 moving columns
    return min((j + 1) // 2, 6)


def _scores_phase(nc, tc, fb, xk, xcol, tts, ptp, dgp, pss):
    pts = {}
    for j in [15] + list(range(15)):
        k0 = 0 if j == 15 else _k0(j)
        ncol = (NQT - k0) * P
        pt = ptp.tile([P, H], BF16)
        pts[j] = pt
        for qoff in range(0, ncol, 512):
            qw = min(512, ncol - qoff)
            st = pss.tile([P, 512], F32, tag="st")
            for dp in range(ND):
                nc.tensor.matmul(
                    st[:, :qw],
                    xk[:, xcol(dp, j):xcol(dp, j) + P],
                    tts[dp][:, k0 * P + qoff:k0 * P + qoff + qw],
                    start=(dp == 0), stop=(dp == ND - 1),
                )
            if j == 15:
                nc.scalar.activation(
                    pt[:, qoff:qoff + qw], st[:, :qw],
                    mybir.ActivationFunctionType.Exp,
                    bias=fb[:, 0:1], scale=SCALE,
                )
            elif qoff == 0:
                dg = dgp.tile([P, 256], F32, tag="dg")
                nc.vector.tensor_copy(dg[:], st[:, 0:256])
                # keep where 256*k0 + 256*kk + f - (128*j + p) >= 0
                nc.gpsimd.affine_select(
                    out=dg[:], in_=dg[:],
                    compare_op=mybir.AluOpType.is_ge,
                    fill=NEG, base=256 * k0 - P * j,
                    pattern=[[256, 2], [1, P]], channel_multiplier=-1,
                )
                nc.scalar.activation(
                    pt[:, 0:256], dg[:],
                    mybir.ActivationFunctionType.Exp,
                    bias=0.0, scale=SCALE,
                )
                if qw > 256:
                    nc.scalar.activation(
                        pt[:, 256:qw], st[:, 256:qw],
                        mybir.ActivationFunctionType.Exp,
                        bias=0.0, scale=SCALE,
                    )
            else:
                nc.scalar.activation(
                    pt[:, qoff:qoff + qw], st[:, :qw],
                    mybir.ActivationFunctionType.Exp,
                    bias=0.0, scale=SCALE,
                )
    return pts


def _build_nc():
    nc = bacc.Bacc("TRN2", target_bir_lowering=False)

    # all inputs host-packed to [128, W] with the 6 d-blocks concatenated
    # along columns -> one DMA descriptor per partition (KB-scale elements)
    m_d = nc.dram_tensor("m", [P, ND * D], BF16, kind="ExternalInput")
    xqA_d = nc.dram_tensor("xqA", [P, ND * 512], BF16, kind="ExternalInput")
    xqB_d = nc.dram_tensor("xqB", [P, ND * 512], BF16, kind="ExternalInput")
    # x keys packed chunk-major: col = 3072*c + 512*d + (j%4)*128, c = j//4
    xk_d = nc.dram_tensor("xk", [P, ND * S], BF16, kind="ExternalInput")
    wvT_d = nc.dram_tensor("wvT", [P, ND * D], BF16, kind="ExternalInput")
    fb_d = nc.dram_tensor("fbias", [P, 1], F32, kind="ExternalInput")
    out_d = nc.dram_tensor("out", [H, D], F32, kind="ExternalOutput")

    with tile.TileContext(nc) as tc:
        with (
            tc.tile_pool(name="cst", bufs=1) as cst,
            tc.tile_pool(name="xp", bufs=1) as xp,
            tc.tile_pool(name="ttp", bufs=ND) as ttp,
            tc.tile_pool(name="vp", bufs=NK) as vp,
            tc.tile_pool(name="ptp", bufs=NK) as ptp,
            tc.tile_pool(name="dgp", bufs=2) as dgp,
            tc.tile_pool(name="sgp", bufs=2) as sgp,
            tc.tile_pool(name="op", bufs=2) as op,
        ):
            fb = cst.tile([P, 1], F32)
            nc.sync.dma_start(out=fb[:], in_=fb_d[:, :])

            tts, vs = [], []
            xk = xp.tile([P, ND * S], BF16)
            # ---- projections (m/xq/wv pools scoped so their SBUF+PSUM free) ----
            with (
                tc.tile_pool(name="mp", bufs=1) as mp,
                tc.tile_pool(name="xqp", bufs=2) as xqp,
                tc.tile_pool(name="wvp", bufs=1) as wvp,
                tc.tile_pool(name="psj", bufs=4, space="PSUM") as psj,
            ):
                # scalar HW queue: TT inputs (critical path); sync HW queue:
                # fb + x key chunks + wv. One big-element DMA each.
                msb = mp.tile([P, ND * D], BF16)
                nc.scalar.dma_start(out=msb[:], in_=m_d[:, :])
                xqA = xqp.tile([P, ND * 512], BF16)
                nc.scalar.dma_start(out=xqA[:], in_=xqA_d[:, :])
                xqB = xqp.tile([P, ND * 512], BF16)
                nc.scalar.dma_start(out=xqB[:], in_=xqB_d[:, :])
                nc.sync.dma_start(out=xk[:, 0:3072], in_=xk_d[:, 0:3072])
                wvsb = wvp.tile([P, ND * D], BF16)
                nc.sync.dma_start(out=wvsb[:], in_=wvT_d[:, :])
                for c in range(1, 4):
                    nc.sync.dma_start(
                        out=xk[:, 3072 * c:3072 * (c + 1)],
                        in_=xk_d[:, 3072 * c:3072 * (c + 1)])

                def xcol(d, j):
                    # column of key tile j's d-block in the chunk-major layout
                    return 3072 * (j // 4) + 512 * d + 128 * (j % 4)

                # TT[d', i] = sum_d M[d, d'] xq^T[d, i]  (qc outer: the first
                # six groups only need m + xqA)
                for et in range(ND):
                    tt = ttp.tile([P, H], BF16)
                    tts.append(tt)
                for qi, xq in enumerate((xqA, xqB)):
                    for et in range(ND):
                        acc = psj.tile([P, 512], F32, tag="ps")
                        for d in range(ND):
                            nc.tensor.matmul(
                                acc[:],
                                msb[:, D * d + et * P:D * d + (et + 1) * P],
                                xq[:, 512 * d:512 * (d + 1)],
                                start=(d == 0), stop=(d == ND - 1),
                            )
                        nc.vector.tensor_copy(tts[et][:, 512 * qi:512 * (qi + 1)], acc[:])

                # V[j, e] = sum_d x^T[d, j] wv^T[d, e]; ones in cols 768..769
                ones = nc.const_aps.tensor(1.0, (P, 2), F32)
                for j in range(NK):
                    v = vp.tile([P, D + 2], BF16)
                    vs.append(v)
                    for e0, ew in ((0, 512), (512, 256)):
                        acc = psj.tile([P, 512], F32, tag="ps")
                        for d in range(ND):
                            nc.tensor.matmul(
                                acc[:, :ew],
                                xk[:, xcol(d, j):xcol(d, j) + P],
                                wvsb[:, D * d + e0:D * d + e0 + ew],
                                start=(d == 0), stop=(d == ND - 1),
                            )
                        nc.vector.tensor_copy(v[:, e0:e0 + ew], acc[:, :ew])
                    nc.vector.tensor_copy(v[:, D:D + 2], ones)

            # ---- attention ----
            # scores^T[j', i] per key tile j: moving = TT[:, 128*k0 :] (query
            # suffix), stationary = x key tile j. Wrap tile 15 first (needed
            # by every PV chain); its exp is gated by fbias. Other groups get
            # one affine_select over the first 256 columns killing the
            # diagonal triangle and any dead leading tile. Then PV per query
            # tile k accumulates over j in {15, 0..2k}; the ones-columns of V
            # give the softmax denominator in pv[:, 768].
            with (
                tc.tile_pool(name="pss", bufs=2, space="PSUM") as pss,
                tc.tile_pool(name="pspv", bufs=2, space="PSUM") as pspv,
            ):
                pts = _scores_phase(nc, tc, fb, xk, xcol, tts, ptp, dgp, pss)

                # k descending: the tail after the last matmul is the
                # shortest chain's divide + store
                for k in range(NQT - 1, -1, -1):
                    pv = pspv.tile([P, D + 2], F32, tag="pv")
                    js = [15] + list(range(2 * k + 1))
                    for idx, j in enumerate(js):
                        k0 = 0 if j == 15 else _k0(j)
                        koff = (k - k0) * P
                        for e0, ew in ((0, 512), (512, D + 2 - 512)):
                            nc.tensor.matmul(
                                pv[:, e0:e0 + ew],
                                pts[j][:, koff:koff + P],
                                vs[j][:, e0:e0 + ew],
                                start=(idx == 0), stop=(idx == len(js) - 1),
                            )
                    rcp = sgp.tile([P, 1], F32, tag="rcp")
                    nc.vector.reciprocal(rcp[:], pv[:, D:D + 1])
                    o = op.tile([P, D], F32, tag="o")
                    nc.vector.tensor_scalar_mul(o[:], pv[:, :D], rcp[:])
                    nc.sync.dma_start(out=out_d[k * P:(k + 1) * P, :], in_=o[:])

    nc.compile()
    return nc


def _get_nc():
    if "nc" not in _cached:
        _cached["nc"] = _build_nc()
    return _cached["nc"]


def kernel(x, w_q, w_k, w_v):
    global last_results
    x = np.ascontiguousarray(np.asarray(x, dtype=np.float32))
    w_q = np.asarray(w_q, dtype=np.float32)
    w_k = np.asarray(w_k, dtype=np.float32)
    w_v = np.asarray(w_v, dtype=np.float32)

    def pack_w(w):
        # [768, 768] -> [128, 6*768] with d-blocks along columns
        return np.ascontiguousarray(
            w.reshape(ND, P, D).transpose(1, 0, 2).reshape(P, ND * D)).astype(BF)

    m = pack_w(w_q.T @ w_k)
    wvT = pack_w(np.ascontiguousarray(w_v.T))

    nc = _get_nc()
    in_maps = []
    for core in range(8):
        b, h = core // 2, core % 2
        r = P * h
        rot = np.concatenate([x[b, r:], x[b, :r]], axis=0)
        xT = np.ascontiguousarray(rot.T)                      # [768, 2048]
        xk = np.ascontiguousarray(
            xT.reshape(ND, P, 4, 512).transpose(1, 2, 0, 3).reshape(P, ND * S)
        ).astype(BF)
        xqT = np.ascontiguousarray(
            x[b].reshape(NK, P, D)[h::2].reshape(H, D).T)     # [768, 1024]
        xqA = np.ascontiguousarray(
            xqT[:, 0:512].reshape(ND, P, 512).transpose(1, 0, 2).reshape(P, ND * 512)
        ).astype(BF)
        xqB = np.ascontiguousarray(
            xqT[:, 512:H].reshape(ND, P, 512).transpose(1, 0, 2).reshape(P, ND * 512)
        ).astype(BF)
        in_maps.append({
            "m": m,
            "xqA": xqA,
            "xqB": xqB,
            "xk": xk,
            "wvT": wvT,
            "fbias": np.full((P, 1), 0.0 if h == 1 else NEG, np.float32),
        })

    trace = bool(int(os.environ.get("KERNEL_TRACE", "0")))
    res = run_bass_kernel_spmd(nc, in_maps, core_ids=list(range(8)), trace=trace)
    last_results = res

    out = np.empty((B, S, D), np.float32)
    for core in range(8):
        b, h = core // 2, core % 2
        o = res.results[core]["out"]
        out[b].reshape(NK, P, D)[h::2] = o.reshape(NQT, P, D)
    return out


# revision 18
# speedup vs baseline: 1.8666x; 1.0393x over previous
"""Causal single-head attention (B=4, S=2048, D=768) on 8 TRN2 NeuronCores.

Sharding: core (b, h) = batch b, query-interleave h. Each core computes the
attention output for query tiles {2k+h : k=0..7} (128 rows each) of one
batch. Keys are fed ROTATED by 128*h so every core sees the identical score
structure: query tile k sits at rotated row 256k and attends rotated key
tiles 0..2k (tile 2k triangular) plus the wrap tile 15, which holds the
original first 128 keys for h=1 and is killed via the exp bias for h=0.
This balances causal work exactly across the core pair.

QK merge: scores = x (W_q^T W_k) x^T, with M = W_q^T W_k computed on the
host. The kernel computes TT = M^T xq^T (one projection instead of Q and K)
and uses the resident x tiles directly as the score stationaries, removing
the K projection from the device entirely.

All matmul inputs are bf16 (1 cycle/row on the PE, same as f32r, but half
the DMA bytes); accumulation stays f32 in PSUM. Softmax denominators come
from two ones-columns appended to V (cols 768..769), divided out on the way
to the output.
"""

import os
import numpy as np
import ml_dtypes

import concourse.bass as bass
import concourse.mybir as mybir
import concourse.tile as tile
from concourse import bacc
from concourse.bass_utils import run_bass_kernel_spmd

B, S, D = 4, 2048, 768
P = 128
ND = D // P          # 6 contraction tiles
NQT = 8              # query tiles per core (128 rows each)
H = NQT * P          # 1024 query rows per core
NK = S // P          # 16 key tiles
SCALE = 1.0 / float(np.sqrt(D))
NEG = -10000.0
F32 = mybir.dt.float32
BF16 = mybir.dt.bfloat16
BF = ml_dtypes.bfloat16

_cached = {}
last_results = None


def _k0(j):
    # first query tile whose score group includes key tile j (j < 15)
    return (j + 1) // 2


def _scores_phase(nc, tc, fb, xk, xcol, tts, ptp, dgp, pss):
    pts = {}
    for j in [15] + list(range(15)):
        k0 = 0 if j == 15 else _k0(j)
        ncol = (NQT - k0) * P
        pt = ptp.tile([P, H], BF16)
        pts[j] = pt
        for qoff in range(0, ncol, 512):
            qw = min(512, ncol - qoff)
            st = pss.tile([P, 512], F32, tag="st")
            for dp in range(ND):
                nc.tensor.matmul(
                    st[:, :qw],
                    xk[:, xcol(dp, j):xcol(dp, j) + P],
                    tts[dp][:, k0 * P + qoff:k0 * P + qoff + qw],
                    start=(dp == 0), stop=(dp == ND - 1),
                )
            if j == 15:
                nc.scalar.activation(
                    pt[:, qoff:qoff + qw], st[:, :qw],
                    mybir.ActivationFunctionType.Exp,
                    bias=fb[:, 0:1], scale=SCALE,
                )
            elif qoff == 0 and j % 2 == 0:
                # even j: leading tile is the diagonal triangle; odd j's
                # leading tile is already strictly below the diagonal
                mw = min(256, ncol)
                dg = dgp.tile([P, 256], F32, tag="dg")
                nc.vector.tensor_copy(dg[:, :mw], st[:, 0:mw])
                # keep where 256*k0 + 256*kk + f - (128*j + p) >= 0
                nc.gpsimd.affine_select(
                    out=dg[:, :mw], in_=dg[:, :mw],
                    compare_op=mybir.AluOpType.is_ge,
                    fill=NEG, base=256 * k0 - P * j,
                    pattern=([[256, 2], [1, P]] if mw == 256 else [[1, P]]),
                    channel_multiplier=-1,
                )
                nc.scalar.activation(
                    pt[:, 0:mw], dg[:, :mw],
                    mybir.ActivationFunctionType.Exp,
                    bias=0.0, scale=SCALE,
                )
                if qw > mw:
                    nc.scalar.activation(
                        pt[:, mw:qw], st[:, mw:qw],
                        mybir.ActivationFunctionType.Exp,
                        bias=0.0, scale=SCALE,
                    )
            else:
                nc.scalar.activation(
                    pt[:, qoff:qoff + qw], st[:, :qw],
                    mybir.ActivationFunctionType.Exp,
                    bias=0.0, scale=SCALE,
                )
    return pts


def _build_nc():
    nc = bacc.Bacc("TRN2", target_bir_lowering=False)

    # all inputs host-packed to [128, W] with the 6 d-blocks concatenated
    # along columns -> one DMA descriptor per partition (KB-scale elements)
    m_d = nc.dram_tensor("m", [P, ND * D], BF16, kind="ExternalInput")
    xqA_d = nc.dram_tensor("xqA", [P, ND * 512], BF16, kind="ExternalInput")
    xqB_d = nc.dram_tensor("xqB", [P, ND * 512], BF16, kind="ExternalInput")
    # x keys packed chunk-major: col = 3072*c + 512*d + (j%4)*128, c = j//4
    xk_d = nc.dram_tensor("xk", [P, ND * S], BF16, kind="ExternalInput")
    wvT_d = nc.dram_tensor("wvT", [P, ND * D], BF16, kind="ExternalInput")
    fb_d = nc.dram_tensor("fbias", [P, 1], F32, kind="ExternalInput")
    out_d = nc.dram_tensor("out", [H, D], F32, kind="ExternalOutput")

    with tile.TileContext(nc) as tc:
        with (
            tc.tile_pool(name="cst", bufs=1) as cst,
            tc.tile_pool(name="xp", bufs=1) as xp,
            tc.tile_pool(name="ttp", bufs=ND) as ttp,
            tc.tile_pool(name="vp", bufs=NK) as vp,
            tc.tile_pool(name="ptp", bufs=NK) as ptp,
            tc.tile_pool(name="dgp", bufs=2) as dgp,
            tc.tile_pool(name="sgp", bufs=2) as sgp,
            tc.tile_pool(name="op", bufs=2) as op,
        ):
            fb = cst.tile([P, 1], F32)
            nc.sync.dma_start(out=fb[:], in_=fb_d[:, :])

            tts, vs = [], []
            xk = xp.tile([P, ND * S], BF16)
            # ---- projections (m/xq/wv pools scoped so their SBUF+PSUM free) ----
            with (
                tc.tile_pool(name="mp", bufs=1) as mp,
                tc.tile_pool(name="xqp", bufs=2) as xqp,
                tc.tile_pool(name="wvp", bufs=1) as wvp,
                tc.tile_pool(name="psj", bufs=4, space="PSUM") as psj,
            ):
                # two HW queues in parallel for the TT critical path:
                # scalar queue streams m in et-chunks (group et needs only
                # chunk et), sync queue delivers xqA first, then keys + wv.
                msb = mp.tile([P, ND * D], BF16)
                for et in range(ND):
                    nc.scalar.dma_start(
                        out=msb[:, D * et:D * (et + 1)],
                        in_=m_d[:, D * et:D * (et + 1)])
                xqB = xqp.tile([P, ND * 512], BF16)
                nc.scalar.dma_start(out=xqB[:], in_=xqB_d[:, :])
                xqA = xqp.tile([P, ND * 512], BF16)
                nc.sync.dma_start(out=xqA[:], in_=xqA_d[:, :])
                nc.sync.dma_start(out=xk[:, 0:3072], in_=xk_d[:, 0:3072])
                wvsb = wvp.tile([P, ND * D], BF16)
                nc.sync.dma_start(out=wvsb[:], in_=wvT_d[:, :])
                for c in range(1, 4):
                    nc.sync.dma_start(
                        out=xk[:, 3072 * c:3072 * (c + 1)],
                        in_=xk_d[:, 3072 * c:3072 * (c + 1)])

                def xcol(d, j):
                    # column of key tile j's d-block in the chunk-major layout
                    return 3072 * (j // 4) + 512 * d + 128 * (j % 4)

                # TT[d', i] = sum_d M[d, d'] xq^T[d, i]  (qc outer: the first
                # six groups only need m + xqA)
                for et in range(ND):
                    tt = ttp.tile([P, H], BF16)
                    tts.append(tt)
                for qi, xq in enumerate((xqA, xqB)):
                    for et in range(ND):
                        acc = psj.tile([P, 512], F32, tag="ps")
                        for d in range(ND):
                            nc.tensor.matmul(
                                acc[:],
                                msb[:, D * et + P * d:D * et + P * (d + 1)],
                                xq[:, 512 * d:512 * (d + 1)],
                                start=(d == 0), stop=(d == ND - 1),
                            )
                        nc.vector.tensor_copy(tts[et][:, 512 * qi:512 * (qi + 1)], acc[:])

                # V[j, e] = sum_d x^T[d, j] wv^T[d, e]; ones in cols 768..769
                ones = nc.const_aps.tensor(1.0, (P, 2), F32)
                for j in range(NK):
                    v = vp.tile([P, D + 2], BF16)
                    vs.append(v)
                    for e0, ew in ((0, 512), (512, 256)):
                        acc = psj.tile([P, 512], F32, tag="ps")
                        for d in range(ND):
                            nc.tensor.matmul(
                                acc[:, :ew],
                                xk[:, xcol(d, j):xcol(d, j) + P],
                                wvsb[:, D * d + e0:D * d + e0 + ew],
                                start=(d == 0), stop=(d == ND - 1),
                            )
                        nc.vector.tensor_copy(v[:, e0:e0 + ew], acc[:, :ew])
                    nc.vector.tensor_copy(v[:, D:D + 2], ones)

            # ---- attention ----
            # scores^T[j', i] per key tile j: moving = TT[:, 128*k0 :] (query
            # suffix), stationary = x key tile j. Wrap tile 15 first (needed
            # by every PV chain); its exp is gated by fbias. Other groups get
            # one affine_select over the first 256 columns killing the
            # diagonal triangle and any dead leading tile. Then PV per query
            # tile k accumulates over j in {15, 0..2k}; the ones-columns of V
            # give the softmax denominator in pv[:, 768].
            with (
                tc.tile_pool(name="pss", bufs=3, space="PSUM") as pss,
                tc.tile_pool(name="pspv", bufs=2, space="PSUM") as pspv,
            ):
                pts = _scores_phase(nc, tc, fb, xk, xcol, tts, ptp, dgp, pss)

                # k descending: the tail after the last matmul is the
                # shortest chain's divide + store
                for k in range(NQT - 1, -1, -1):
                    pv = pspv.tile([P, D + 2], F32, tag="pv")
                    js = [15] + list(range(2 * k + 1))
                    for idx, j in enumerate(js):
                        k0 = 0 if j == 15 else _k0(j)
                        koff = (k - k0) * P
                        for e0, ew in ((0, 512), (512, D + 2 - 512)):
                            nc.tensor.matmul(
                                pv[:, e0:e0 + ew],
                                pts[j][:, koff:koff + P],
                                vs[j][:, e0:e0 + ew],
                                start=(idx == 0), stop=(idx == len(js) - 1),
                            )
                    rcp = sgp.tile([P, 1], F32, tag="rcp")
                    nc.vector.reciprocal(rcp[:], pv[:, D:D + 1])
                    o = op.tile([P, D], F32, tag="o")
                    nc.vector.tensor_scalar_mul(o[:], pv[:, :D], rcp[:])
                    nc.sync.dma_start(out=out_d[k * P:(k + 1) * P, :], in_=o[:])

    nc.compile()
    return nc


def _get_nc():
    if "nc" not in _cached:
        _cached["nc"] = _build_nc()
    return _cached["nc"]


def kernel(x, w_q, w_k, w_v):
    global last_results
    x = np.ascontiguousarray(np.asarray(x, dtype=np.float32))
    w_q = np.asarray(w_q, dtype=np.float32)
    w_k = np.asarray(w_k, dtype=np.float32)
    w_v = np.asarray(w_v, dtype=np.float32)

    def pack_w(w):
        # [768, 768] -> [128, 6*768] with d-blocks along columns
        return np.ascontiguousarray(
            w.reshape(ND, P, D).transpose(1, 0, 2).reshape(P, ND * D)).astype(BF)

    # m packed et-major: col = 768*et + 128*d + c  ->  M[128d+p, 128et+c]
    m = np.ascontiguousarray(
        (w_q.T @ w_k).reshape(ND, P, ND, P).transpose(1, 2, 0, 3).reshape(P, ND * D)
    ).astype(BF)
    wvT = pack_w(np.ascontiguousarray(w_v.T))

    nc = _get_nc()
    in_maps = []
    for core in range(8):
        b, h = core // 2, core % 2
        r = P * h
        rot = np.concatenate([x[b, r:], x[b, :r]], axis=0)
        xT = np.ascontiguousarray(rot.T)                      # [768, 2048]
        xk = np.ascontiguousarray(
            xT.reshape(ND, P, 4, 512).transpose(1, 2, 0, 3).reshape(P, ND * S)
        ).astype(BF)
        xqT = np.ascontiguousarray(
            x[b].reshape(NK, P, D)[h::2].reshape(H, D).T)     # [768, 1024]
        xqA = np.ascontiguousarray(
            xqT[:, 0:512].reshape(ND, P, 512).transpose(1, 0, 2).reshape(P, ND * 512)
        ).astype(BF)
        xqB = np.ascontiguousarray(
            xqT[:, 512:H].reshape(ND, P, 512).transpose(1, 0, 2).reshape(P, ND * 512)
        ).astype(BF)
        in_maps.append({
            "m": m,
            "xqA": xqA,
            "xqB": xqB,
            "xk": xk,
            "wvT": wvT,
            "fbias": np.full((P, 1), 0.0 if h == 1 else NEG, np.float32),
        })

    trace = bool(int(os.environ.get("KERNEL_TRACE", "0")))
    res = run_bass_kernel_spmd(nc, in_maps, core_ids=list(range(8)), trace=trace)
    last_results = res

    out = np.empty((B, S, D), np.float32)
    for core in range(8):
        b, h = core // 2, core % 2
        o = res.results[core]["out"]
        out[b].reshape(NK, P, D)[h::2] = o.reshape(NQT, P, D)
    return out


# revision 21
# speedup vs baseline: 1.9627x; 1.0515x over previous
"""Causal single-head attention (B=4, S=2048, D=768) on 8 TRN2 NeuronCores.

Sharding: core (b, h) = batch b, query-interleave h. Each core computes the
attention output for query tiles {2k+h : k=0..7} (128 rows each) of one
batch. Keys are fed ROTATED by 128*h so every core sees the identical score
structure: query tile k sits at rotated row 256k and attends rotated key
tiles 0..2k (tile 2k triangular) plus the wrap tile 15, which holds the
original first 128 keys for h=1 and is killed via the exp bias for h=0.
This balances causal work exactly across the core pair.

QK merge: scores = x (W_q^T W_k) x^T, with M = W_q^T W_k computed on the
host. The kernel computes TT = M^T xq^T (one projection instead of Q and K)
and uses the resident x tiles directly as the score stationaries, removing
the K projection from the device entirely.

All matmul inputs are bf16 (1 cycle/row on the PE, same as f32r, but half
the DMA bytes); accumulation stays f32 in PSUM. Softmax denominators come
from two ones-columns appended to V (cols 768..769), divided out on the way
to the output.
"""

import os
import numpy as np
import ml_dtypes

import concourse.bass as bass
import concourse.mybir as mybir
import concourse.tile as tile
from concourse import bacc
from concourse.bass_utils import run_bass_kernel_spmd

B, S, D = 4, 2048, 768
P = 128
ND = D // P          # 6 contraction tiles
NQT = 8              # query tiles per core (128 rows each)
H = NQT * P          # 1024 query rows per core
NK = S // P          # 16 key tiles
SCALE = 1.0 / float(np.sqrt(D))
NEG = -10000.0
F32 = mybir.dt.float32
BF16 = mybir.dt.bfloat16
BF = ml_dtypes.bfloat16

_cached = {}
last_results = None


def _k0(j):
    # first query tile whose score group includes key tile j (j < 15)
    return (j + 1) // 2


def _scores_phase(nc, tc, fb, xk, xcol, tts, ptp, dgp, pss):
    pts = {}
    for j in [15] + list(range(15)):
        k0 = 0 if j == 15 else _k0(j)
        ncol = (NQT - k0) * P
        pt = ptp.tile([P, H], BF16)
        pts[j] = pt
        for qoff in range(0, ncol, 512):
            qw = min(512, ncol - qoff)
            st = pss.tile([P, 512], F32, tag="st")
            for dp in range(ND):
                nc.tensor.matmul(
                    st[:, :qw],
                    xk[:, xcol(dp, j):xcol(dp, j) + P],
                    tts[dp][:, k0 * P + qoff:k0 * P + qoff + qw],
                    start=(dp == 0), stop=(dp == ND - 1),
                )
            if j == 15:
                nc.scalar.activation(
                    pt[:, qoff:qoff + qw], st[:, :qw],
                    mybir.ActivationFunctionType.Exp,
                    bias=fb[:, 0:1], scale=SCALE,
                )
            elif qoff == 0 and j % 2 == 0:
                # even j: leading tile is the diagonal triangle; odd j's
                # leading tile is already strictly below the diagonal
                mw = min(256, ncol)
                dg = dgp.tile([P, 256], F32, tag="dg")
                nc.vector.tensor_copy(dg[:, :mw], st[:, 0:mw])
                # keep where 256*k0 + 256*kk + f - (128*j + p) >= 0
                nc.gpsimd.affine_select(
                    out=dg[:, :mw], in_=dg[:, :mw],
                    compare_op=mybir.AluOpType.is_ge,
                    fill=NEG, base=256 * k0 - P * j,
                    pattern=([[256, 2], [1, P]] if mw == 256 else [[1, P]]),
                    channel_multiplier=-1,
                )
                nc.scalar.activation(
                    pt[:, 0:mw], dg[:, :mw],
                    mybir.ActivationFunctionType.Exp,
                    bias=0.0, scale=SCALE,
                )
                if qw > mw:
                    nc.scalar.activation(
                        pt[:, mw:qw], st[:, mw:qw],
                        mybir.ActivationFunctionType.Exp,
                        bias=0.0, scale=SCALE,
                    )
            else:
                nc.scalar.activation(
                    pt[:, qoff:qoff + qw], st[:, :qw],
                    mybir.ActivationFunctionType.Exp,
                    bias=0.0, scale=SCALE,
                )
    return pts


def _build_nc():
    nc = bacc.Bacc("TRN2", target_bir_lowering=False)

    # all inputs host-packed to [128, W] with the 6 d-blocks concatenated
    # along columns -> one DMA descriptor per partition (KB-scale elements)
    m_d = nc.dram_tensor("m", [P, ND * D], BF16, kind="ExternalInput")
    xqA_d = nc.dram_tensor("xqA", [P, ND * 512], BF16, kind="ExternalInput")
    xqB_d = nc.dram_tensor("xqB", [P, ND * 512], BF16, kind="ExternalInput")
    # x keys packed chunk-major: col = 3072*c + 512*d + (j%4)*128, c = j//4
    xk_d = nc.dram_tensor("xk", [P, ND * S], BF16, kind="ExternalInput")
    # x keys again in [j, d] row layout + two ones columns, per-tile blocks
    x2_d = nc.dram_tensor("x2", [P, NK * (D + 2)], BF16, kind="ExternalInput")
    wvT_d = nc.dram_tensor("wvT", [P, ND * D], BF16, kind="ExternalInput")
    fb_d = nc.dram_tensor("fbias", [P, 1], F32, kind="ExternalInput")
    out_d = nc.dram_tensor("out", [H, D], F32, kind="ExternalOutput")

    with tile.TileContext(nc) as tc:
        with (
            tc.tile_pool(name="cst", bufs=1) as cst,
            tc.tile_pool(name="xp", bufs=1) as xp,
            tc.tile_pool(name="x2p", bufs=1) as x2p,
            tc.tile_pool(name="wvp", bufs=1) as wvp,
            tc.tile_pool(name="ttp", bufs=ND) as ttp,
            tc.tile_pool(name="ptp", bufs=NK) as ptp,
            tc.tile_pool(name="dgp", bufs=2) as dgp,
            tc.tile_pool(name="sgp", bufs=2) as sgp,
            tc.tile_pool(name="pxp", bufs=3) as pxp,
            tc.tile_pool(name="pxtp", bufs=2) as pxtp,
            tc.tile_pool(name="op", bufs=2) as op,
        ):
            fb = cst.tile([P, 1], F32)
            nc.sync.dma_start(out=fb[:], in_=fb_d[:, :])

            tts = []
            xk = xp.tile([P, ND * S], BF16)
            x2 = x2p.tile([P, NK * (D + 2)], BF16)
            wvsb = wvp.tile([P, ND * D], BF16)
            # ---- TT projection (m/xq pools scoped so their SBUF+PSUM free) ----
            with (
                tc.tile_pool(name="mp", bufs=1) as mp,
                tc.tile_pool(name="xqp", bufs=2) as xqp,
                tc.tile_pool(name="psj", bufs=4, space="PSUM") as psj,
            ):
                # two HW queues in parallel for the TT critical path:
                # scalar queue streams m in et-chunks (group et needs only
                # chunk et) then xqB, x2, wv; sync queue delivers xqA then
                # the key chunks in scores-consumption order (c3 first).
                msb = mp.tile([P, ND * D], BF16)
                for et in range(ND):
                    nc.scalar.dma_start(
                        out=msb[:, D * et:D * (et + 1)],
                        in_=m_d[:, D * et:D * (et + 1)])
                xqB = xqp.tile([P, ND * 512], BF16)
                nc.scalar.dma_start(out=xqB[:], in_=xqB_d[:, :])
                xqA = xqp.tile([P, ND * 512], BF16)
                nc.sync.dma_start(out=xqA[:], in_=xqA_d[:, :])
                for c in (3, 0, 1, 2):
                    nc.sync.dma_start(
                        out=xk[:, 3072 * c:3072 * (c + 1)],
                        in_=xk_d[:, 3072 * c:3072 * (c + 1)])
                for half in range(2):
                    w = NK * (D + 2) // 2
                    nc.scalar.dma_start(
                        out=x2[:, w * half:w * (half + 1)],
                        in_=x2_d[:, w * half:w * (half + 1)])
                nc.scalar.dma_start(out=wvsb[:], in_=wvT_d[:, :])

                def xcol(d, j):
                    # column of key tile j's d-block in the chunk-major layout
                    return 3072 * (j // 4) + 512 * d + 128 * (j % 4)

                # TT[d', i] = sum_d M[d, d'] xq^T[d, i]  (qc outer: the first
                # six groups only need m + xqA)
                for et in range(ND):
                    tt = ttp.tile([P, H], BF16)
                    tts.append(tt)
                for qi, xq in enumerate((xqA, xqB)):
                    for et in range(ND):
                        acc = psj.tile([P, 512], F32, tag="ps")
                        for d in range(ND):
                            nc.tensor.matmul(
                                acc[:],
                                msb[:, D * et + P * d:D * et + P * (d + 1)],
                                xq[:, 512 * d:512 * (d + 1)],
                                start=(d == 0), stop=(d == ND - 1),
                            )
                        nc.vector.tensor_copy(tts[et][:, 512 * qi:512 * (qi + 1)], acc[:])

            # ---- scores + exp -> PT tiles (own PSUM scope) ----
            with tc.tile_pool(name="pss", bufs=3, space="PSUM") as pss:
                pts = _scores_phase(nc, tc, fb, xk, xcol, tts, ptp, dgp, pss)

            # ---- Px = P @ [x | 1 1] per query tile k, then out = Pxn @ wv^T.
            # The ones columns give the softmax denominator in px[:, 768];
            # the divide lands on the bf16 Pxn copy; the d<->i transpose for
            # the final contraction runs on the DMA xbar, not the PE. fin(k)
            # is scheduled after px(k-1) so the divide+transpose latency of
            # px(k) hides under the px(k-1) matmul chain.
            with (
                tc.tile_pool(name="ppx", bufs=2, space="PSUM") as ppx,
                tc.tile_pool(name="pfin", bufs=1, space="PSUM") as pfin,
            ):
                def px_chain(k):
                    px = ppx.tile([P, D + 2], F32, tag="px")
                    js = [15] + list(range(2 * k + 1))
                    for idx, j in enumerate(js):
                        k0 = 0 if j == 15 else _k0(j)
                        koff = (k - k0) * P
                        for e0, ew in ((0, 512), (512, D + 2 - 512)):
                            nc.tensor.matmul(
                                px[:, e0:e0 + ew],
                                pts[j][:, koff:koff + P],
                                x2[:, (D + 2) * j + e0:(D + 2) * j + e0 + ew],
                                start=(idx == 0), stop=(idx == len(js) - 1),
                            )
                    rcp = sgp.tile([P, 1], F32, tag="rcp")
                    nc.vector.reciprocal(rcp[:], px[:, D:D + 1])
                    pxn = pxp.tile([P, D], BF16, tag="pxn")
                    nc.vector.tensor_scalar_mul(pxn[:], px[:, :D], rcp[:])
                    pxt = pxtp.tile([P, ND, P], BF16, tag="pxt")
                    nc.sync.dma_start_transpose(out=pxt[:], in_=pxn[:])
                    return pxt

                def fin_chain(k, pxt):
                    fin = pfin.tile([P, D], F32, tag="fin")
                    for di in range(ND):
                        for e0, ew in ((0, 512), (512, 256)):
                            nc.tensor.matmul(
                                fin[:, e0:e0 + ew],
                                pxt[:, di, :],
                                wvsb[:, D * di + e0:D * di + e0 + ew],
                                start=(di == 0), stop=(di == ND - 1),
                            )
                    o = op.tile([P, D], F32, tag="o")
                    nc.vector.tensor_copy(o[:], fin[:])
                    nc.sync.dma_start(out=out_d[k * P:(k + 1) * P, :], in_=o[:])

                pxts = {}
                pxts[7] = px_chain(7)
                for k in range(6, -1, -1):
                    pxts[k] = px_chain(k)
                    fin_chain(k + 1, pxts.pop(k + 1))
                fin_chain(0, pxts.pop(0))

    nc.compile()
    return nc


def _get_nc():
    if "nc" not in _cached:
        _cached["nc"] = _build_nc()
    return _cached["nc"]


def kernel(x, w_q, w_k, w_v):
    global last_results
    x = np.ascontiguousarray(np.asarray(x, dtype=np.float32))
    w_q = np.asarray(w_q, dtype=np.float32)
    w_k = np.asarray(w_k, dtype=np.float32)
    w_v = np.asarray(w_v, dtype=np.float32)

    def pack_w(w):
        # [768, 768] -> [128, 6*768] with d-blocks along columns
        return np.ascontiguousarray(
            w.reshape(ND, P, D).transpose(1, 0, 2).reshape(P, ND * D)).astype(BF)

    # m packed et-major: col = 768*et + 128*d + c  ->  M[128d+p, 128et+c]
    m = np.ascontiguousarray(
        (w_q.T @ w_k).reshape(ND, P, ND, P).transpose(1, 2, 0, 3).reshape(P, ND * D)
    ).astype(BF)
    wvT = pack_w(np.ascontiguousarray(w_v.T))

    nc = _get_nc()
    in_maps = []
    for core in range(8):
        b, h = core // 2, core % 2
        r = P * h
        rot = np.concatenate([x[b, r:], x[b, :r]], axis=0)
        xT = np.ascontiguousarray(rot.T)                      # [768, 2048]
        xk = np.ascontiguousarray(
            xT.reshape(ND, P, 4, 512).transpose(1, 2, 0, 3).reshape(P, ND * S)
        ).astype(BF)
        xqT = np.ascontiguousarray(
            x[b].reshape(NK, P, D)[h::2].reshape(H, D).T)     # [768, 1024]
        xqA = np.ascontiguousarray(
            xqT[:, 0:512].reshape(ND, P, 512).transpose(1, 0, 2).reshape(P, ND * 512)
        ).astype(BF)
        xqB = np.ascontiguousarray(
            xqT[:, 512:H].reshape(ND, P, 512).transpose(1, 0, 2).reshape(P, ND * 512)
        ).astype(BF)
        x2 = np.ascontiguousarray(
            np.concatenate([rot, np.ones((S, 2), np.float32)], axis=1)
            .reshape(NK, P, D + 2).transpose(1, 0, 2).reshape(P, NK * (D + 2))
        ).astype(BF)
        in_maps.append({
            "m": m,
            "xqA": xqA,
            "xqB": xqB,
            "xk": xk,
            "x2": x2,
            "wvT": wvT,
            "fbias": np.full((P, 1), 0.0 if h == 1 else NEG, np.float32),
        })

    trace = bool(int(os.environ.get("KERNEL_TRACE", "0")))
    res = run_bass_kernel_spmd(nc, in_maps, core_ids=list(range(8)), trace=trace)
    last_results = res

    out = np.empty((B, S, D), np.float32)
    for core in range(8):
        b, h = core // 2, core % 2
        o = res.results[core]["out"]
        out[b].reshape(NK, P, D)[h::2] = o.reshape(NQT, P, D)
    return out


# revision 24
# speedup vs baseline: 2.0059x; 1.0220x over previous
"""Causal single-head attention (B=4, S=2048, D=768) on 8 TRN2 NeuronCores.

Sharding: core (b, h) = batch b, query-interleave h. Each core computes the
attention output for query tiles {2k+h : k=0..7} (128 rows each) of one
batch. Keys are fed ROTATED by 128*h so every core sees the identical score
structure: query tile k sits at rotated row 256k and attends rotated key
tiles 0..2k (tile 2k triangular) plus the wrap tile 15, which holds the
original first 128 keys for h=1 and is killed via the exp bias for h=0.
This balances causal work exactly across the core pair.

QK merge: scores = x (W_q^T W_k) x^T, with M = W_q^T W_k computed on the
host. The kernel computes TT = M^T xq^T (one projection instead of Q and K)
and uses the resident x tiles directly as the score stationaries, removing
the K projection from the device entirely.

All matmul inputs are bf16 (1 cycle/row on the PE, same as f32r, but half
the DMA bytes); accumulation stays f32 in PSUM. Softmax denominators come
from two ones-columns appended to V (cols 768..769), divided out on the way
to the output.
"""

import os
import numpy as np
import ml_dtypes

import concourse.bass as bass
import concourse.mybir as mybir
import concourse.tile as tile
from concourse import bacc
from concourse.bass_utils import run_bass_kernel_spmd

B, S, D = 4, 2048, 768
P = 128
ND = D // P          # 6 contraction tiles
NQT = 8              # query tiles per core (128 rows each)
H = NQT * P          # 1024 query rows per core
NK = S // P          # 16 key tiles
SCALE = 1.0 / float(np.sqrt(D))
NEG = -10000.0
F32 = mybir.dt.float32
BF16 = mybir.dt.bfloat16
BF = ml_dtypes.bfloat16

_cached = {}
last_results = None


def _k0(j):
    # first query tile whose score group includes key tile j (j < 15)
    return (j + 1) // 2


def _scores_phase(nc, tc, fb, xk, xcol, tts, ptp, dgp, pss):
    pts = {}
    for j in [15] + list(range(15)):
        k0 = 0 if j == 15 else _k0(j)
        ncol = (NQT - k0) * P
        pt = ptp.tile([P, H], BF16)
        pts[j] = pt
        for qoff in range(0, ncol, 512):
            qw = min(512, ncol - qoff)
            st = pss.tile([P, 512], F32, tag="st")
            for dp in range(ND):
                nc.tensor.matmul(
                    st[:, :qw],
                    xk[:, xcol(dp, j):xcol(dp, j) + P],
                    tts[dp][:, k0 * P + qoff:k0 * P + qoff + qw],
                    start=(dp == 0), stop=(dp == ND - 1),
                )
            if j == 15:
                nc.scalar.activation(
                    pt[:, qoff:qoff + qw], st[:, :qw],
                    mybir.ActivationFunctionType.Exp,
                    bias=fb[:, 0:1], scale=SCALE,
                )
            elif qoff == 0 and j % 2 == 0:
                # even j: leading tile is the diagonal triangle; odd j's
                # leading tile is already strictly below the diagonal
                mw = min(256, ncol)
                dg = dgp.tile([P, 256], F32, tag="dg")
                nc.vector.tensor_copy(dg[:, :mw], st[:, 0:mw])
                # keep where 256*k0 + 256*kk + f - (128*j + p) >= 0
                nc.gpsimd.affine_select(
                    out=dg[:, :mw], in_=dg[:, :mw],
                    compare_op=mybir.AluOpType.is_ge,
                    fill=NEG, base=256 * k0 - P * j,
                    pattern=([[256, 2], [1, P]] if mw == 256 else [[1, P]]),
                    channel_multiplier=-1,
                )
                nc.scalar.activation(
                    pt[:, 0:mw], dg[:, :mw],
                    mybir.ActivationFunctionType.Exp,
                    bias=0.0, scale=SCALE,
                )
                if qw > mw:
                    nc.scalar.activation(
                        pt[:, mw:qw], st[:, mw:qw],
                        mybir.ActivationFunctionType.Exp,
                        bias=0.0, scale=SCALE,
                    )
            else:
                nc.scalar.activation(
                    pt[:, qoff:qoff + qw], st[:, :qw],
                    mybir.ActivationFunctionType.Exp,
                    bias=0.0, scale=SCALE,
                )
    return pts


def _build_nc():
    nc = bacc.Bacc("TRN2", target_bir_lowering=False)

    # all inputs host-packed to [128, W] with the 6 d-blocks concatenated
    # along columns -> one DMA descriptor per partition (KB-scale elements)
    m_d = nc.dram_tensor("m", [P, ND * D], BF16, kind="ExternalInput")
    xqA_d = nc.dram_tensor("xqA", [P, ND * 512], BF16, kind="ExternalInput")
    xqB_d = nc.dram_tensor("xqB", [P, ND * 512], BF16, kind="ExternalInput")
    # x keys packed chunk-major: col = 3072*c + 512*d + (j%4)*128, c = j//4
    xk_d = nc.dram_tensor("xk", [P, ND * S], BF16, kind="ExternalInput")
    # x keys again in [j, d] row layout + two ones columns, per-tile blocks
    x2_d = nc.dram_tensor("x2", [P, NK * (D + 2)], BF16, kind="ExternalInput")
    wvT_d = nc.dram_tensor("wvT", [P, ND * D], BF16, kind="ExternalInput")
    fb_d = nc.dram_tensor("fbias", [P, 1], F32, kind="ExternalInput")
    out_d = nc.dram_tensor("out", [H, D], F32, kind="ExternalOutput")

    with tile.TileContext(nc) as tc:
        with (
            tc.tile_pool(name="cst", bufs=1) as cst,
            tc.tile_pool(name="xp", bufs=1) as xp,
            tc.tile_pool(name="x2p", bufs=1) as x2p,
            tc.tile_pool(name="wvp", bufs=1) as wvp,
            tc.tile_pool(name="ttp", bufs=ND) as ttp,
            tc.tile_pool(name="ptp", bufs=NK) as ptp,
            tc.tile_pool(name="dgp", bufs=2) as dgp,
            tc.tile_pool(name="sgp", bufs=2) as sgp,
            tc.tile_pool(name="pxp", bufs=3) as pxp,
            tc.tile_pool(name="pxtp", bufs=NQT) as pxtp,
            tc.tile_pool(name="op", bufs=2) as op,
        ):
            fb = cst.tile([P, 1], F32)
            nc.sync.dma_start(out=fb[:], in_=fb_d[:, :])

            tts = []
            xk = xp.tile([P, ND * S], BF16)
            x2 = x2p.tile([P, NK * (D + 2)], BF16)
            wvsb = wvp.tile([P, ND * D], BF16)
            # ---- TT projection (m/xq pools scoped so their SBUF+PSUM free) ----
            with (
                tc.tile_pool(name="mp", bufs=1) as mp,
                tc.tile_pool(name="xqp", bufs=2) as xqp,
                tc.tile_pool(name="psj", bufs=4, space="PSUM") as psj,
            ):
                # two HW queues in parallel for the TT critical path:
                # scalar queue streams m in et-chunks (group et needs only
                # chunk et) then xqB, x2, wv; sync queue delivers xqA then
                # the key chunks in scores-consumption order (c3 first).
                msb = mp.tile([P, ND * D], BF16)
                for et in range(ND):
                    nc.scalar.dma_start(
                        out=msb[:, D * et:D * (et + 1)],
                        in_=m_d[:, D * et:D * (et + 1)])
                xqB = xqp.tile([P, ND * 512], BF16)
                nc.scalar.dma_start(out=xqB[:], in_=xqB_d[:, :])
                xqA = xqp.tile([P, ND * 512], BF16)
                nc.sync.dma_start(out=xqA[:], in_=xqA_d[:, :])
                for c in (3, 0, 1, 2):
                    nc.sync.dma_start(
                        out=xk[:, 3072 * c:3072 * (c + 1)],
                        in_=xk_d[:, 3072 * c:3072 * (c + 1)])
                for half in range(2):
                    w = NK * (D + 2) // 2
                    nc.scalar.dma_start(
                        out=x2[:, w * half:w * (half + 1)],
                        in_=x2_d[:, w * half:w * (half + 1)])
                nc.scalar.dma_start(out=wvsb[:], in_=wvT_d[:, :])

                def xcol(d, j):
                    # column of key tile j's d-block in the chunk-major layout
                    return 3072 * (j // 4) + 512 * d + 128 * (j % 4)

                # TT[d', i] = sum_d M[d, d'] xq^T[d, i]  (qc outer: the first
                # six groups only need m + xqA)
                for et in range(ND):
                    tt = ttp.tile([P, H], BF16)
                    tts.append(tt)
                for qi, xq in enumerate((xqA, xqB)):
                    for et in range(ND):
                        acc = psj.tile([P, 512], F32, tag="ps")
                        for d in range(ND):
                            nc.tensor.matmul(
                                acc[:],
                                msb[:, D * et + P * d:D * et + P * (d + 1)],
                                xq[:, 512 * d:512 * (d + 1)],
                                start=(d == 0), stop=(d == ND - 1),
                            )
                        nc.vector.tensor_copy(tts[et][:, 512 * qi:512 * (qi + 1)], acc[:])

            # ---- scores + exp -> PT tiles (own PSUM scope) ----
            with tc.tile_pool(name="pss", bufs=3, space="PSUM") as pss:
                pts = _scores_phase(nc, tc, fb, xk, xcol, tts, ptp, dgp, pss)

            # ---- Px = P @ [x | 1 1] per query tile k, then out = Pxn @ wv^T.
            # The ones columns give the softmax denominator in px[:, 768];
            # the divide lands on the bf16 Pxn copy; the d<->i transpose for
            # the final contraction runs on the DMA xbar, not the PE. fin(k)
            # is scheduled after px(k-1) so the divide+transpose latency of
            # px(k) hides under the px(k-1) matmul chain.
            with (
                tc.tile_pool(name="ppx", bufs=3, space="PSUM") as ppx,
                tc.tile_pool(name="pfin", bufs=1, space="PSUM") as pfin,
            ):
                def px_chain(k):
                    px = ppx.tile([P, D + 2], F32, tag="px")
                    js = [15] + list(range(2 * k + 1))
                    for idx, j in enumerate(js):
                        k0 = 0 if j == 15 else _k0(j)
                        koff = (k - k0) * P
                        for e0, ew in ((0, 512), (512, D + 2 - 512)):
                            nc.tensor.matmul(
                                px[:, e0:e0 + ew],
                                pts[j][:, koff:koff + P],
                                x2[:, (D + 2) * j + e0:(D + 2) * j + e0 + ew],
                                start=(idx == 0), stop=(idx == len(js) - 1),
                            )
                    rcp = sgp.tile([P, 1], F32, tag="rcp")
                    nc.vector.reciprocal(rcp[:], px[:, D:D + 1])
                    pxn = pxp.tile([P, D], BF16, tag="pxn")
                    nc.vector.tensor_scalar_mul(pxn[:], px[:, :D], rcp[:])
                    pxt = pxtp.tile([P, ND, P], BF16, tag="pxt")
                    nc.sync.dma_start_transpose(out=pxt[:], in_=pxn[:])
                    return pxt

                def fin_chain(k, pxt):
                    fin = pfin.tile([P, D], F32, tag="fin")
                    for di in range(ND):
                        for e0, ew in ((0, 512), (512, 256)):
                            nc.tensor.matmul(
                                fin[:, e0:e0 + ew],
                                pxt[:, di, :],
                                wvsb[:, D * di + e0:D * di + e0 + ew],
                                start=(di == 0), stop=(di == ND - 1),
                            )
                    o = op.tile([P, D], F32, tag="o")
                    nc.vector.tensor_copy(o[:], fin[:])
                    nc.sync.dma_start(out=out_d[k * P:(k + 1) * P, :], in_=o[:])

                # all px chains first (divides + xbar transposes trail on
                # Vector/DMA), then all fin chains - by fin time every pxt
                # is ready, so the PE never waits on the transpose latency
                pxts = {k: px_chain(k) for k in range(NQT - 1, -1, -1)}
                for k in range(NQT - 1, -1, -1):
                    fin_chain(k, pxts.pop(k))

    nc.compile()
    return nc


def _get_nc():
    if "nc" not in _cached:
        _cached["nc"] = _build_nc()
    return _cached["nc"]


def kernel(x, w_q, w_k, w_v):
    global last_results
    x = np.ascontiguousarray(np.asarray(x, dtype=np.float32))
    w_q = np.asarray(w_q, dtype=np.float32)
    w_k = np.asarray(w_k, dtype=np.float32)
    w_v = np.asarray(w_v, dtype=np.float32)

    def pack_w(w):
        # [768, 768] -> [128, 6*768] with d-blocks along columns
        return np.ascontiguousarray(
            w.reshape(ND, P, D).transpose(1, 0, 2).reshape(P, ND * D)).astype(BF)

    # m packed et-major: col = 768*et + 128*d + c  ->  M[128d+p, 128et+c]
    m = np.ascontiguousarray(
        (w_q.T @ w_k).reshape(ND, P, ND, P).transpose(1, 2, 0, 3).reshape(P, ND * D)
    ).astype(BF)
    wvT = pack_w(np.ascontiguousarray(w_v.T))

    nc = _get_nc()
    in_maps = []
    for core in range(8):
        b, h = core // 2, core % 2
        r = P * h
        rot = np.concatenate([x[b, r:], x[b, :r]], axis=0)
        xT = np.ascontiguousarray(rot.T)                      # [768, 2048]
        xk = np.ascontiguousarray(
            xT.reshape(ND, P, 4, 512).transpose(1, 2, 0, 3).reshape(P, ND * S)
        ).astype(BF)
        xqT = np.ascontiguousarray(
            x[b].reshape(NK, P, D)[h::2].reshape(H, D).T)     # [768, 1024]
        xqA = np.ascontiguousarray(
            xqT[:, 0:512].reshape(ND, P, 512).transpose(1, 0, 2).reshape(P, ND * 512)
        ).astype(BF)
        xqB = np.ascontiguousarray(
            xqT[:, 512:H].reshape(ND, P, 512).transpose(1, 0, 2).reshape(P, ND * 512)
        ).astype(BF)
        x2 = np.ascontiguousarray(
            np.concatenate([rot, np.ones((S, 2), np.float32)], axis=1)
            .reshape(NK, P, D + 2).transpose(1, 0, 2).reshape(P, NK * (D + 2))
        ).astype(BF)
        in_maps.append({
            "m": m,
            "xqA": xqA,
            "xqB": xqB,
            "xk": xk,
            "x2": x2,
            "wvT": wvT,
            "fbias": np.full((P, 1), 0.0 if h == 1 else NEG, np.float32),
        })

    trace = bool(int(os.environ.get("KERNEL_TRACE", "0")))
    res = run_bass_kernel_spmd(nc, in_maps, core_ids=list(range(8)), trace=trace)
    last_results = res

    out = np.empty((B, S, D), np.float32)
    for core in range(8):
        b, h = core // 2, core % 2
        o = res.results[core]["out"]
        out[b].reshape(NK, P, D)[h::2] = o.reshape(NQT, P, D)
    return out
